# revision 56
# baseline (speedup 1.0000x reference)
"""Trainium2 Bass kernel for nn_CPCircuitLayer (embedding_lookup).

Math: A_b = X_b @ W_seq^T [S,R]; Bm_b = X_b^T @ W_hid^T [H,R]
      out[b, n] = dot(A_b[idx_s[n]], Bm_b[idx_h[n]]),  out -> [B, S, H]

Key reformulation: out[b, n] = G_b[idx_s[n], idx_h[n]] where
G_b = A_b @ Bm_b^T is a [S, H] = [1024, 1024] matrix that fits in SBUF
(tiny matmul chain: ~100M MACs total). The problem becomes a scalar
gather of N entries from G per batch.

Device layout: both batches' tables interleave in one SBUF tile
tab[p, 2e+b] bf16 ([128, 8192, 2]; partition p = s'%128,
e = (s'//128)*1024 + h), so a single d=2 ap_gather index fetches BOTH
batches' values for a cell. ap_gather costs ~27.4ns per index per
16-partition group (measured; independent of d/num_elems/value order,
but idx slices must stay 4B-aligned), so index count is everything.

Index-count reduction vs the naive 131072/group:
  * 2 outputs/index via the batch interleave (d=2)
  * host dedup of repeated (s, h) cells (~6%)
  * a gathered column delivers a cell to ALL 16 lanes of its group, so
    only DISTINCT (group, e) pairs need gathering. At this problem's
    density (~1.9 wanted cells per (group, e)) that halves indices
  * the host clusters rows with overlapping h-sets into the same
    (group, slot) cell (greedy min-union-growth bitset packing),
    shrinking the distinct-(group, e) unions another ~9% -> 6492 max
    per group (LG=6560 padded), 20x fewer than naive.

Sharding: core c handles n in [c*N/8, (c+1)*N/8) for both batches and
computes factors + G for its (host-)row-permuted X; the permutation
(with W_hid columns permuted to match) implements the clustering and
balances groups and table halves. X^T for the A-factor is sent
pre-transposed by the host (contiguous DMA; transpose-DMA and
PE-transpose are both slower).

The table is split into two SBUF tiles (e < 4096 / e >= 4096): phase-1
gather rounds depend only on G blocks k=0..3, so gathering starts
~12us before the PE finishes writing the table (tile-granular deps do
the overlap automatically). Each round's raw [128, rc, 2] bf16 block
DMAs straight to DRAM. The host resolves the hardware-forced residual
(which of the 16 lanes holds each output's row, duplicate expansion,
inverse permutation) with one fancy index per core. Rare stream
overflow (inputs with pathological cell distributions) falls back to
exact host evaluation for the few dropped outputs.

NOTE: keep the idx (isb) DMA issued FIRST on its queue — ap_gather's
idx-stream dependency appears weakly tracked, and issuing it behind
the 4 MB of x^T chunks produced corrupted gathers.

Measured on trn2: ~235,000 ns NEFF exec (NTFF, +-3us run-to-run),
rel rms 4.4e-3 (vs 3,677,069 ns for the staged baseline = 15.6x).
Budget: ~48us to phase-1 gather launch (PE-bound; factor-A's second
half is an independent accumulation chain deferred until after G-lo,
hidden under phase-1 gathers) + ~181us gather (6560 idx/group) + ~6us
tail. Run-to-run exec_time noise is ~+-3us.
"""

import numpy as np
import ml_dtypes
from contextlib import ExitStack

import concourse.bass as bass
import concourse.mybir as mybir
import concourse.tile as tile
from concourse import bacc

B, S, H, R = 2, 1024, 1024, 32
N = S * H
NCORES = 8
J0 = N // NCORES          # 131072 n-indices per core (serves both batches)
NEH = 4 * 1024            # d=2 table blocks per half-table partition row
# The table is split into two SBUF tiles (e < 4096 / e >= 4096) so
# phase-1 gathers depend only on G blocks k=0..3: gathering starts
# while the PE still writes the second half of the table.
LG_LO = 3296              # padded lo-stream length per group (max 3280
LG_HI = 3264              # and 3236 with the overlap-clustered row
                          # assignment below; deterministic per input)
LG = LG_LO + LG_HI        # 6560
RND_LO = (1024, 1024, 1024, LG_LO - 3072)
RND_HI = (1024, 1024, 1024, LG_HI - 3072)
OUTW = 2 * LG             # out cols per core: [128, OUTW] bf16
IDXC = LG // 16           # 452 idx cols per partition

F32 = mybir.dt.float32
BF16 = mybir.dt.bfloat16
I16 = mybir.dt.int16


def _build(reps: int = 1):
    nc = bacc.Bacc()
    x0 = nc.declare_dram_parameter("x0", [S, H], BF16, False)
    x1 = nc.declare_dram_parameter("x1", [S, H], BF16, False)
    xt0 = nc.declare_dram_parameter("xt0", [H, S], BF16, False)
    xt1 = nc.declare_dram_parameter("xt1", [H, S], BF16, False)
    # already row-scattered on host: [p, k, r] = W^T[p + 128k, r]
    wseq_t = nc.declare_dram_parameter("wseq_t", [128, 8 * R], BF16, False)
    whid_t = nc.declare_dram_parameter("whid_t", [128, 8 * R], BF16, False)
    idx = nc.declare_dram_parameter("idx", [128, IDXC], I16, False)
    out = nc.declare_dram_parameter("out", [128, OUTW], BF16, True)
    xs = (x0, x1)
    xts = (xt0, xt1)

    with tile.TileContext(nc) as tc, ExitStack() as ctx:
        base = ctx.enter_context(tc.tile_pool(name="base", bufs=1))
        fps = ctx.enter_context(tc.tile_pool(name="fps", bufs=2, space="PSUM"))
        gps = ctx.enter_context(tc.tile_pool(name="gps", bufs=2, space="PSUM"))
        tabp = ctx.enter_context(tc.tile_pool(name="tabp", bufs=1))
        facp = ctx.enter_context(tc.tile_pool(name="facp", bufs=1))
        gap = ctx.enter_context(tc.tile_pool(name="gap", bufs=2))

        # --- static loads -----------------------------------------------
        ws_sb = base.tile([128, 8, R], BF16)     # W_seq^T rows, h-major
        wh_sb = base.tile([128, 8, R], BF16)     # W_hid^T rows, s-major
        isb = base.tile([128, IDXC], I16)
        x_sb = base.tile([128, 2, 8, H], BF16)   # [p, b, k, h]; s' = p + 128k
        xt_sb = base.tile([128, 2, 8, S], BF16)  # [p, b, c, s]; h = p + 128c

        # sync queue in first-use order: wh (first factor's lhsT), then
        # the x0 chunks it streams against, then ws (needed ~15us later),
        # then x1. idx stays FIRST on the ACT queue (see NOTE above),
        # followed by the host-pretransposed x^T chunks.
        nc.scalar.dma_start(out=isb[:], in_=idx[:])
        nc.sync.dma_start(out=wh_sb[:], in_=whid_t[:])
        for b in range(B):
            for k in range(8):
                nc.sync.dma_start(
                    out=x_sb[:, b, k, :],
                    in_=bass.AP(tensor=xs[b][:].tensor, offset=128 * k * H,
                                ap=[[H, 128], [1, H]]),
                )
                nc.scalar.dma_start(
                    out=xt_sb[:, b, k, :],
                    in_=bass.AP(tensor=xts[b][:].tensor, offset=128 * k * S,
                                ap=[[S, 128], [1, S]]),
                )
            if b == 0:
                nc.sync.dma_start(out=ws_sb[:], in_=wseq_t[:])

        for _ in range(reps):
            _body(nc, fps, gps, tabp, facp, gap,
                  ws_sb, wh_sb, isb, x_sb, xt_sb, out)
    nc.compile()
    return nc


def _body(nc, fps, gps, tabp, facp, gap,
          ws_sb, wh_sb, isb, x_sb, xt_sb, out):
    tab_lo = tabp.tile([128, 2 * NEH], BF16, tag="tab_lo")  # e < 4096
    tab_hi = tabp.tile([128, 2 * NEH], BF16, tag="tab_hi")  # e >= 4096
    a_bf = facp.tile([32, 2, S], BF16, tag="a_bf")   # A_b^T[r, s']
    b_bf = facp.tile([32, 2, H], BF16, tag="b_bf")   # Bm_b^T[r, h]

    # --- factors: B fully, A half-0 only (G-lo needs a_bf cols 0-511
    # and all of b_bf; A half-1 is an independent accumulation chain
    # and computes later, hidden under the phase-1 gathers) -----------
    for b in range(B):
        ptb = fps.tile([R, 1024], F32, tag="ptb")
        for k in range(8):
            for nh in range(2):
                nc.tensor.matmul(
                    out=ptb[:, nh * 512:(nh + 1) * 512],
                    lhsT=wh_sb[:, k, :],
                    rhs=x_sb[:, b, k, nh * 512:(nh + 1) * 512],
                    start=(k == 0), stop=(k == 7),
                )
        nc.scalar.copy(out=b_bf[:, b, :], in_=ptb[:])
        pta = fps.tile([R, 512], F32, tag="pta")
        for k in range(8):
            nc.tensor.matmul(
                out=pta[:],
                lhsT=ws_sb[:, k, :],
                rhs=xt_sb[:, b, k, 0:512],
                start=(k == 0), stop=(k == 7),
            )
        nc.vector.tensor_copy(out=a_bf[:, b, 0:512], in_=pta[:])

    def g_blocks(krange, eng):
        # block k covers s' in [128k, 128k+128): out partition i = s'-128k,
        # half-table col e' = 1024*(k%4) + h, written at tab[:, 2e' + b].
        for k in krange:
            tabt = tab_lo if k < 4 else tab_hi
            for b in range(B):
                for nh in range(2):
                    gp = gps.tile([128, 512], F32, tag="gp")
                    nc.tensor.matmul(
                        out=gp[:],
                        lhsT=a_bf[:, b, 128 * k:128 * (k + 1)],
                        rhs=b_bf[:, b, 512 * nh:512 * (nh + 1)],
                        start=True, stop=True,
                    )
                    dst = bass.AP(
                        tensor=tabt[:].tensor,
                        offset=tabt[:].offset + 2 * (1024 * (k % 4) + 512 * nh) + b,
                        ap=[list(tabt[:].ap[0]), [2, 512]],
                    )
                    if eng % 2 == 0:
                        nc.vector.tensor_copy(out=dst, in_=gp[:])
                    else:
                        nc.scalar.copy(out=dst, in_=gp[:])
                    eng += 1
        return eng

    eng = g_blocks(range(4), 0)          # tab_lo complete -> phase 1 go

    # factor A half-1, then the hi table blocks (overlap phase-1 gather)
    for b in range(B):
        pta = fps.tile([R, 512], F32, tag="pta")
        for k in range(8):
            nc.tensor.matmul(
                out=pta[:],
                lhsT=ws_sb[:, k, :],
                rhs=xt_sb[:, b, k, 512:1024],
                start=(k == 0), stop=(k == 7),
            )
        nc.vector.tensor_copy(out=a_bf[:, b, 512:1024], in_=pta[:])
    g_blocks(range(4, 8), eng)

    # --- gather + writeback --------------------------------------------
    # phase 1 reads only tab_lo and starts while the PE writes tab_hi;
    # raw [128, rc, 2] bf16 blocks DMA straight to DRAM for host-side
    # (lane, position) resolution.
    col = 0
    outoff = 0
    for (tabt, rounds) in ((tab_lo, RND_LO), (tab_hi, RND_HI)):
        tab_flat = bass.AP(tensor=tabt[:].tensor, offset=tabt[:].offset,
                           ap=[list(tabt[:].ap[0]), [1, 2 * NEH], [1, 1]])
        for rc in rounds:
            ga = gap.tile([128, 2048], BF16, tag="ga")
            ga_ap = bass.AP(tensor=ga[:].tensor, offset=ga[:].offset,
                            ap=[list(ga[:].ap[0]), [1, 2 * rc], [1, 1]])
            nc.gpsimd.ap_gather(
                out_ap=ga_ap, in_ap=tab_flat,
                idxs_ap=isb[:, col:col + rc // 16],
                channels=128, num_elems=NEH, d=2, num_idxs=rc,
            )
            nc.sync.dma_start(
                out=bass.AP(tensor=out[:].tensor, offset=outoff,
                            ap=[[OUTW, 128], [1, 2 * rc]]),
                in_=ga[:, :2 * rc],
            )
            col += rc // 16
            outoff += 2 * rc


_nc_cache_by_reps = {}


def _get_nc(reps: int = 1):
    nc = _nc_cache_by_reps.get(reps)
    if nc is None:
        nc = _nc_cache_by_reps[reps] = _build(reps)
    return nc


class _Runner:
    """Trace/compile the SPMD executable once; reuse across calls."""

    def __init__(self, nc):
        import jax
        from jax.experimental.shard_map import shard_map
        from jax.sharding import Mesh, PartitionSpec
        import concourse.bass2jax as b2j

        b2j.install_neuronx_cc_hook()
        self.nc = nc
        part_name = (nc.partition_id_tensor.name
                     if nc.partition_id_tensor else None)
        in_names, out_names, out_avals = [], [], []
        zero_outs = []
        for alloc in nc.m.functions[0].allocations:
            if not isinstance(alloc, mybir.MemoryLocationSet):
                continue
            name = alloc.memorylocations[0].name
            if alloc.kind == "ExternalInput":
                if name != part_name:
                    in_names.append(name)
            elif alloc.kind == "ExternalOutput":
                out_names.append(name)
                shape = tuple(alloc.tensor_shape)
                dtype = mybir.dt.np(alloc.dtype)
                out_avals.append(jax.core.ShapedArray(shape, dtype))
                zero_outs.append(np.zeros(shape, dtype))
        self.in_names = list(in_names)
        self.out_names = out_names
        self.zero_outs = zero_outs
        n_params = len(in_names)
        n_outs = len(out_names)
        all_in_names = in_names + out_names
        if part_name is not None:
            all_in_names = all_in_names + [part_name]
        donate = tuple(range(n_params, n_params + n_outs))

        def _body_fn(*args):
            operands = list(args)
            if part_name is not None:
                operands.append(b2j.partition_id_tensor())
            outs = b2j._bass_exec_p.bind(
                *operands,
                out_avals=tuple(out_avals),
                in_names=tuple(all_in_names),
                out_names=tuple(out_names),
                lowering_input_output_aliases=(),
                sim_require_finite=True,
                sim_require_nnan=True,
                nc=nc,
            )
            return tuple(outs)

        devices = jax.devices()[:NCORES]
        mesh = Mesh(np.asarray(devices), ("core",))
        self.fn = jax.jit(
            shard_map(
                _body_fn, mesh=mesh,
                in_specs=(PartitionSpec("core"),) * (n_params + n_outs),
                out_specs=(PartitionSpec("core"),) * n_outs,
                check_rep=False,
            ),
            donate_argnums=donate,
            keep_unused=True,
        )

    def __call__(self, in_maps):
        concat_in = [
            np.concatenate([np.asarray(m[name]) for m in in_maps], axis=0)
            for name in self.in_names
        ]
        concat_zeros = [
            np.zeros((NCORES * z.shape[0], *z.shape[1:]), z.dtype)
            for z in self.zero_outs
        ]
        out_arrs = self.fn(*concat_in, *concat_zeros)
        return [
            {
                name: np.asarray(out_arrs[i]).reshape(NCORES, -1)[c]
                for i, name in enumerate(self.out_names)
            }
            for c in range(NCORES)
        ]


_runner_cache = {}


def _get_runner(reps: int = 1):
    r = _runner_cache.get(reps)
    if r is None:
        r = _runner_cache[reps] = _Runner(_get_nc(reps))
    return r


_PAIR_PERM = np.array([0, 2, 1, 3], np.int64)


def _cluster_sigma(us: np.ndarray, uh: np.ndarray) -> np.ndarray:
    """Assign the 1024 s-rows to the 128x8 (partition, slot) grid.

    Greedily clusters rows with overlapping h-sets into 16-row cells
    (one cell = one (group, slot): its distinct-e load is the union of
    its rows' h-sets), then balances cells across groups and across the
    lo/hi table halves. ~8% fewer distinct (group, e) gather indices
    than overlap-blind assignment. Returns sigma: sigma[s'] = original
    row at permuted position s'."""
    bs = np.zeros((S, 16), np.uint64)
    np.bitwise_or.at(bs, (us, uh >> 6), np.uint64(1) << (uh & 63).astype(np.uint64))
    cnt = np.bitwise_count(bs).sum(1)
    remaining = np.ones(S, bool)
    cells, sizes = [], []
    for _ in range(64):
        cand = np.flatnonzero(remaining)
        seed = cand[np.argmax(cnt[cand])]
        rows = [seed]
        remaining[seed] = False
        union = bs[seed].copy()
        for _ in range(15):
            cand = np.flatnonzero(remaining)
            inter = np.bitwise_count(bs[cand] & union).sum(1)
            # min union growth beats max intersection by ~1.4% here
            pick = cand[np.argmin(cnt[cand] - inter)]
            rows.append(pick)
            remaining[pick] = False
            union |= bs[pick]
        cells.append(rows)
        sizes.append(int(np.bitwise_count(union).sum()))
    sizes = np.array(sizes)
    gtot = np.zeros(8, np.int64)
    gcells = [[] for _ in range(8)]
    for ci in np.argsort(-sizes):
        gg = [g for g in range(8) if len(gcells[g]) < 8]
        gpick = min(gg, key=lambda g: gtot[g])
        gcells[gpick].append(ci)
        gtot[gpick] += sizes[ci]
    sigma = np.empty(S, np.int64)
    for g in range(8):
        lo = hi = nlo = nhi = 0
        for ci in sorted(gcells[g], key=lambda ci: -sizes[ci]):
            if (lo <= hi and nlo < 4) or nhi >= 4:
                j = nlo
                nlo += 1
                lo += sizes[ci]
            else:
                j = 4 + nhi
                nhi += 1
                hi += sizes[ci]
            for i, row in enumerate(cells[ci]):
                sigma[(16 * g + i) + 128 * j] = row
    return sigma


def _prep_core(s: np.ndarray, h: np.ndarray):
    """Dedup to distinct (group, e) gather entries for one core's J0
    (s, h) pairs, split into lo/hi table phases.
    Returns (sigma, idx_dev [128, IDXC] int16, meta)."""
    ukey, inv = np.unique(s * 1024 + h, return_inverse=True)
    us, uh = ukey >> 10, ukey & 1023
    sigma = _cluster_sigma(us, uh)
    invpos = np.empty(S, np.int64)
    invpos[sigma] = np.arange(S)
    usp = invpos[us]                     # permuted row position s'
    up = usp & 127                       # partition (lane = up % 16)
    ug = up >> 4                         # group
    ue = ((usp >> 7) << 10) | uh         # (s'//128)*1024 + h < 8192
    # distinct (group, e); order by (group, phase, e) so the two
    # phases form contiguous per-group streams. e-SORTED order makes
    # the ucode's paired 2-count read-APs use small strides
    dge = np.unique(ug * 8192 + ue)
    dg, de = dge >> 13, dge & 8191
    ph = (de >= 4096).astype(np.int64)
    order = np.argsort((dg * 2 + ph) * 8192 + de, kind="stable")
    dg, de, ph = dg[order], de[order], ph[order]
    cnt = np.bincount(dg * 2 + ph, minlength=16)
    starts = np.concatenate(([0], np.cumsum(cnt)[:-1]))
    pos = np.arange(len(dge)) - starts[dg * 2 + ph]
    keepd = pos < np.where(ph == 0, LG_LO, LG_HI)
    streampos = np.where(ph == 0, pos, LG_LO + pos)
    # the Q7 ucode pairs stream positions (4t, 4t+2) and (4t+1, 4t+3)
    # into 2-count read-APs; permuting each sorted 4-tuple to (a,c,b,d)
    # gives both APs adjacent-sorted (small) strides
    streampos = (streampos & ~3) + _PAIR_PERM[streampos & 3]
    val = np.where(ph == 0, de, de - 4096)
    arr = np.zeros((8, LG), np.int16)
    arr[dg[keepd], streampos[keepd]] = val[keepd].astype(np.int16)
    idx_dev = np.empty((128, IDXC), np.int16)
    for g in range(8):
        idx_dev[16 * g:16 * (g + 1), :] = arr[g].reshape(IDXC, 16).T
    # per unique cell: its (partition, stream position)
    lut = np.full((8, 8192), -1, np.int64)
    lut[dg[keepd], de[keepd]] = streampos[keepd]
    pos_u = lut[ug, ue]                  # -1 if its (g, e) was dropped
    return sigma, np.ascontiguousarray(idx_dev), (inv, up, pos_u)


def prepare_in_maps(hidden_states, W_seq, W_hid, all_indices):
    hidden_states = np.asarray(hidden_states)
    all_indices = np.asarray(all_indices)
    x_bf = [hidden_states[b].astype(ml_dtypes.bfloat16) for b in range(B)]
    # W^T row-scattered to the device layout [p, k, r] = W^T[p + 128k, r]
    ws_t = np.ascontiguousarray(
        np.asarray(W_seq).T.astype(ml_dtypes.bfloat16)
        .reshape(8, 128, R).transpose(1, 0, 2).reshape(128, 8 * R))
    wh_t_full = np.asarray(W_hid).T.astype(ml_dtypes.bfloat16)  # [S, R]
    in_maps, metas = [], []
    for c in range(NCORES):
        sl = slice(c * J0, (c + 1) * J0)
        s = all_indices[sl, 0].astype(np.int64)
        h = all_indices[sl, 1].astype(np.int64)
        sigma, idx_dev, meta = _prep_core(s, h)
        metas.append(meta)
        xp = [np.ascontiguousarray(x_bf[b][sigma]) for b in range(B)]
        in_maps.append({
            "x0": xp[0], "x1": xp[1],
            "xt0": np.ascontiguousarray(xp[0].T),
            "xt1": np.ascontiguousarray(xp[1].T),
            "wseq_t": ws_t,
            "whid_t": np.ascontiguousarray(
                wh_t_full[sigma]
                .reshape(8, 128, R).transpose(1, 0, 2).reshape(128, 8 * R)),
            "idx": idx_dev,
        })
    return in_maps, metas


def _assemble(results, metas, hidden_states, all_indices):
    out_full = np.empty((B, N), dtype=np.float32)
    spill = []
    for c in range(NCORES):
        inv, up, pos_u = metas[c]
        dev = np.asarray(results[c]["out"]).reshape(128, LG, 2)
        keep = pos_u >= 0
        uvals = np.zeros((len(up), 2), np.float32)
        uvals[keep] = dev[up[keep], pos_u[keep], :].astype(np.float32)
        vals = uvals[inv]                             # [J0, 2]
        out_full[0, c * J0:(c + 1) * J0] = vals[:, 0]
        out_full[1, c * J0:(c + 1) * J0] = vals[:, 1]
        if not keep.all():
            spill.append(c * J0 + np.flatnonzero(~keep[inv]))
    if spill:
        # exact host fallback for (astronomically rare) stream overflow
        ns = np.concatenate(spill)
        s = all_indices[ns, 0].astype(np.int64)
        h = all_indices[ns, 1].astype(np.int64)
        for b in range(B):
            A = hidden_states[b].astype(np.float32) @ np.asarray(
                _W_CACHE["W_seq"]).T.astype(np.float32)
            Bm = hidden_states[b].astype(np.float32).T @ np.asarray(
                _W_CACHE["W_hid"]).T.astype(np.float32)
            out_full[b, ns] = np.sum(A[s] * Bm[h], axis=-1)
    return out_full.reshape(B, S, H)


_W_CACHE = {}


def kernel(hidden_states, W_seq, W_hid, all_indices):
    hidden_states = np.asarray(hidden_states)
    W_seq = np.asarray(W_seq)
    W_hid = np.asarray(W_hid)
    all_indices = np.asarray(all_indices)
    _W_CACHE["W_seq"] = W_seq
    _W_CACHE["W_hid"] = W_hid

    runner = _get_runner()
    in_maps, metas = prepare_in_maps(hidden_states, W_seq, W_hid, all_indices)
    results = runner(in_maps)
    return _assemble(results, metas, hidden_states, all_indices)
